# revision 21
# baseline (speedup 1.0000x reference)
"""Trainium2 Bass kernel: 16-head self-attention (B=2, N=2048, C=1024) on 8 cores.

Sharding: core c -> (batch b = c//4, head-group g = c%4 owning heads 4g..4g+3).
Each core computes QKV projection for its heads, full softmax attention, and a
partial out-projection (its heads' input-channel slice of W_out); the host sums
the 4 partials per batch (tensor-parallel all-reduce done on host at gather).

v2 design (vs the first working version, which measured 427us):
  - Head PAIRS processed together with PE row-tiling: the two heads of a pair
    keep their 64 d-channels on partitions 0-63 / 64-127, so their K=64 score
    matmuls run CONCURRENTLY on the two row halves of the PE array.
  - Query-block-major loop (4 blocks of 512 queries): the out-projection of
    block b runs inside block b+1's attention window instead of a serial tail.
  - Softmax denominator reciprocals: [1,512] rows are spread to [128,8] via a
    DRAM round-trip so the DVE reciprocal microcode runs across 128 lanes
    (~0.1us) instead of serializing in one lane (6.5us per call in v1).
  - o accumulators are evacuated PSUM->SBUF immediately (one DVE copy) so the
    single accumulator bank pair can be reused by the next head pair at once;
    normalization (broadcast-matmul + multiply) happens later off SBUF.
  - Emission is software-pipelined: PV runs SKEW slots behind scores/exp, and
    projection / out-projection / normalization quanta are placed in fixed
    slots so the Scalar engine's exp stream (the 128us floor) never starves
    and the PE never idles long enough to re-throttle (HAM).
  - exp on ScalarE reads S^T from PSUM as [128,1024] tiles (head A's 512
    queries | head B's 512): P = exp(scale*S + shift); the constant shift
    cancels in normalization.  All matmuls in float32r (full rate at N>=256);
    probabilities and V in bf16.
"""
import os

import numpy as np

B, N, C, H, D = 2, 2048, 1024, 16, 64
HPC = 4            # heads per core
SCALE = float(D) ** -0.5
SHIFT = -12.0      # exp arg shift; scores observed in [-9.1, 9.1] scaled
P = 128
KT = N // 128      # 16 key tiles
QB = N // 512      # 4 query blocks
SKEW = 4           # PV emission lag (slots) behind scores/exp

_cache = {}


def _build_nc():
    import concourse.bass as bass  # noqa: F401
    import concourse.mybir as mybir
    from concourse import bacc
    from concourse.tile import TileContext

    f32 = mybir.dt.float32
    f32r = mybir.dt.float32r
    bf16 = mybir.dt.bfloat16
    Exp = mybir.ActivationFunctionType.Exp
    mult = mybir.AluOpType.mult

    nc = bacc.Bacc("TRN2", target_bir_lowering=False, debug=False, num_devices=8)

    xT = nc.dram_tensor("xT", [P, 8, N], bf16, kind="ExternalInput")
    wqk = nc.dram_tensor("wqk", [P, 8, 512], bf16, kind="ExternalInput")
    wv = nc.dram_tensor("wv", [P, 8, 256], bf16, kind="ExternalInput")
    wo = nc.dram_tensor("wo", [P, 2, 1024], bf16, kind="ExternalInput")
    bqk = nc.dram_tensor("bqk", [P, 4], f32, kind="ExternalInput")
    out_y = nc.dram_tensor("out_y", [N, C], bf16, kind="ExternalOutput")

    with TileContext(nc) as tc:
        with tc.tile_pool(name="sb", bufs=1) as sb, \
             tc.tile_pool(name="ps", bufs=1, space="PSUM") as ps:
            # ---- persistent SBUF ----
            wqk_sb = sb.tile([P, 8, 512], bf16)
            wv_sb = sb.tile([P, 8, 256], bf16)
            wo_sb = sb.tile([P, 2, 1024], bf16)
            bqk_sb = sb.tile([P, 4], f32)
            xT_sb = sb.tile([P, 8, N], bf16)
            qkT_sb = sb.tile([P, 4, N], bf16)       # [q01|q23|k01|k23] x tokens
            v_sb = sb.tile([P, KT, HPC, 65], bf16)  # tokens x (head, D|ones)
            oT_sb = sb.tile([P, 2, N], bf16)        # head channels x tokens
            dS = sb.tile([P, 8], f32)               # denoms spread over lanes
            rS = sb.tile([P, 8], f32r)              # reciprocals, spread
            r2 = sb.tile([1, 1024], f32r)           # reciprocals, heads on free

            # Each DMA trigger costs ~1us of sequencer time, so inputs load
            # as few transfers as possible, split across the SP / Activation /
            # GPSIMD queues so the ramp-critical ones issue in parallel.
            nc.sync.dma_start(wqk_sb[:, 0:4, :], wqk[:, 0:4, :])
            nc.scalar.dma_start(xT_sb[:, 0:4, 0:512], xT[:, 0:4, 0:512])
            nc.gpsimd.dma_start(bqk_sb, bqk[:])
            nc.sync.dma_start(wqk_sb[:, 4:8, :], wqk[:, 4:8, :])
            nc.scalar.dma_start(xT_sb[:, 4:8, 0:512], xT[:, 4:8, 0:512])
            nc.sync.dma_start(xT_sb[:, :, 512:1024], xT[:, :, 512:1024])
            nc.scalar.dma_start(wv_sb, wv[:])
            nc.sync.dma_start(xT_sb[:, :, 1024:2048], xT[:, :, 1024:2048])
            nc.sync.dma_start(wo_sb, wo[:])

            ones_f = sb.tile([P, 1], f32)
            nc.vector.memset(ones_f, 1.0)
            with nc.allow_low_precision(reason="exact 1.0 to bf16"):
                nc.vector.tensor_copy(v_sb[:, :, :, 64:65],
                                      ones_f[:, 0:1, None].to_broadcast((P, KT, HPC, 1)))
            ones_r = sb.tile([1, 64], f32r)
            with nc.allow_low_precision(reason="exact 1.0 to f32r"):
                nc.vector.tensor_copy(ones_r, ones_f[0:1, :].to_broadcast((1, 64)))
            shift_sb = sb.tile([P, 1], f32)
            nc.vector.memset(shift_sb, SHIFT)

            # ---- PE slack fillers ----
            qk_open = {}

            def qk_half(ms, nt, half):
                # K=1024 contraction in 8 steps; emitted as two 4-step halves
                # so a chain never monopolizes the PE for >1us.
                tok = slice(nt * 512, (nt + 1) * 512)
                if half == 0:
                    qk_open[(ms, nt)] = ps.tile([P, 512], f32, tag="c", bufs=1,
                                                name="qk_ps")
                qk_ps = qk_open[(ms, nt)]
                for ks in range(4 * half, 4 * half + 4):
                    nc.tensor.matmul(
                        qk_ps,
                        lhsT=wqk_sb[:, ks, ms * 128:(ms + 1) * 128],
                        rhs=xT_sb[:, ks, tok],
                        start=(ks == 0), stop=(ks == 7),
                    )
                if half == 1:
                    with nc.allow_low_precision(reason="qkT f32r for PE"):
                        nc.vector.tensor_scalar_add(
                            qkT_sb[:, ms, tok], qk_ps, bqk_sb[:, ms:ms + 1])
                    del qk_open[(ms, nt)]

            def qk_chain(ms, nt):
                qk_half(ms, nt, 0)
                qk_half(ms, nt, 1)

            def v_chain(kt):
                v_ps = ps.tile([P, 512], f32, tag="y", bufs=1,
                               name="v_ps")[:, 0:256]
                for ks in range(8):
                    nc.tensor.matmul(
                        v_ps,
                        lhsT=xT_sb[:, ks, kt * 128:(kt + 1) * 128],
                        rhs=wv_sb[:, ks, :],
                        start=(ks == 0), stop=(ks == 7),
                    )
                with nc.allow_low_precision(reason="v bf16 for PE"):
                    nc.vector.tensor_copy(
                        v_sb[:, kt, :, 0:64],
                        v_ps.rearrange("p (h d) -> p h d", h=HPC))

            def op_sub(qb, n, tag="y"):
                # out-projection for one (128-token, 512-outs) tile of block qb
                qt, n2 = n // 2, n % 2
                tok = slice(qb * 512 + qt * 128, qb * 512 + (qt + 1) * 128)
                if tag == "y":
                    y_ps = ps.tile([P, 512], f32, tag="y", bufs=1, name="y_ps")
                else:  # tail: the freed double-buffered scores pool pipelines
                    y_ps = ps.tile([P, 1024], f32, tag="s", bufs=2,
                                   name="s_ps")[:, 0:512]
                for ks2 in range(2):
                    nc.tensor.matmul(
                        y_ps,
                        lhsT=oT_sb[:, ks2, tok],
                        rhs=wo_sb[:, ks2, n2 * 512:(n2 + 1) * 512],
                        start=(ks2 == 0), stop=(ks2 == 1),
                    )
                y_sb = sb.tile([P, 512], bf16, tag="ysb", bufs=2, name="y_sb")
                with nc.allow_low_precision(reason="partial y summed on host"):
                    nc.vector.tensor_copy(y_sb, y_ps)
                nc.sync.dma_start(out_y[tok, n2 * 512:(n2 + 1) * 512], y_sb)

            # ---- evacuation + deferred normalization ----
            def evac(o_ps, eng=None):
                # free the accumulator banks with PSUM->SBUF copies, spreading
                # the denominator row across 128 lanes via SBUF->SBUF DMA so
                # the reciprocal microcode runs in parallel.  Body boundaries
                # use the idle GPSIMD queue; the final evac uses the Scalar
                # queue (idle once the exp stream has finished).
                eng = eng or nc.gpsimd
                dT = sb.tile([1, 1024], f32, tag="dT", bufs=2, name="dT")
                nc.vector.tensor_copy(dT, o_ps[64:65, :])
                eng.dma_start(dS, dT)
                oS = sb.tile([64, 1024], f32, tag="oS", bufs=2, name="oS")
                nc.vector.tensor_copy(oS, o_ps[0:64, :])
                with nc.allow_low_precision(reason="softmax denom recip"):
                    nc.vector.reciprocal(rS, dS)
                eng.dma_start(r2, rS)
                return oS

            def rbm(p, qb, oS, j):
                # broadcast 1/denom along the 64 channel partitions via a
                # ones-matmul, then normalize head j of pair p into oT.
                rb_ps = ps.tile([P, 512], f32, tag="y", bufs=1,
                                name="rb_ps")[0:64, :]
                nc.tensor.matmul(rb_ps, lhsT=ones_r,
                                 rhs=r2[0:1, j * 512:(j + 1) * 512],
                                 start=True, stop=True)
                rbc_sb = sb.tile([64, 512], f32, tag="rbc", bufs=2, name="rbc_sb")
                nc.vector.tensor_copy(rbc_sb, rb_ps)
                with nc.allow_low_precision(reason="oT bf16 for PE"):
                    nc.vector.tensor_tensor(
                        out=oT_sb[64 * j:64 * j + 64, p,
                                  qb * 512:(qb + 1) * 512],
                        in0=oS[0:64, j * 512:(j + 1) * 512],
                        in1=rbc_sb,
                        op=mult,
                    )

            # ---- fixed fill schedule: slot (qb, p, kt) -> PE slack work ----
            fills = {}

            def F(qb, p, kt, fn):
                fills.setdefault((qb, p, kt), []).append(fn)

            for kt in range(KT):
                F(0, 0, kt, lambda kt=kt: v_chain(kt))
            for (ms, nt), s0 in [((2, 1), 3), ((2, 2), 6), ((2, 3), 10),
                                 ((3, 0), 12), ((1, 0), 14)]:
                F(0, 0, s0, lambda ms=ms, nt=nt: qk_half(ms, nt, 0))
                F(0, 0, s0 + 1, lambda ms=ms, nt=nt: qk_half(ms, nt, 1))
            for (ms, nt), s0 in [((3, 1), 3), ((3, 2), 5), ((3, 3), 9),
                                 ((0, 1), 12)]:
                F(0, 1, s0, lambda ms=ms, nt=nt: qk_half(ms, nt, 0))
                F(0, 1, s0 + 1, lambda ms=ms, nt=nt: qk_half(ms, nt, 1))
            for qb in range(1, QB):
                for n in range(4):
                    F(qb, 0, 10 + n, lambda qb=qb, n=n: op_sub(qb - 1, n))
                F(qb, 0, 14, lambda qb=qb: qk_half(1, qb, 0))
                F(qb, 0, 15, lambda qb=qb: qk_half(1, qb, 1))
                for n, s in [(4, 9), (5, 10), (6, 11), (7, 12)]:
                    F(qb, 1, s, lambda qb=qb, n=n: op_sub(qb - 1, n))
                if qb < QB - 1:
                    F(qb, 1, 13, lambda qb=qb: qk_half(0, qb + 1, 0))
                    F(qb, 1, 14, lambda qb=qb: qk_half(0, qb + 1, 1))

            # ---- attention machinery ----
            def scores_exp(qb, p, kt, pend, o_ps):
                key = slice(kt * 128, (kt + 1) * 128)
                q = slice(qb * 512, (qb + 1) * 512)
                s_ps = ps.tile([P, 1024], f32, tag="s", bufs=2, name="s_ps")
                for j in range(2):  # j: head 2p+j on PE rows 64j..64j+63
                    hp = 64 * j
                    nc.tensor.matmul(
                        s_ps[:, j * 512:(j + 1) * 512],
                        lhsT=qkT_sb[hp:hp + 64, 2 + p, key],
                        rhs=qkT_sb[hp:hp + 64, p, q],
                        start=True, stop=True,
                    )
                pT = sb.tile([P, 1024], bf16, tag="pT", bufs=8, name="pT")
                nc.scalar.activation(pT, s_ps, Exp, bias=shift_sb, scale=SCALE)
                pend.append((kt, pT, o_ps, p))

            def pv(pend):
                kt, pT, o_ps, p = pend.pop(0)
                for j in range(2):
                    nc.tensor.matmul(
                        o_ps[0:65, j * 512:(j + 1) * 512],
                        lhsT=v_sb[:, kt, 2 * p + j, :],
                        rhs=pT[:, j * 512:(j + 1) * 512],
                        start=(kt == 0), stop=(kt == KT - 1),
                    )

            # ramp: minimal chains for (0,0) slot 0
            qk_chain(0, 0)
            qk_chain(2, 0)

            pend = []
            prev = None       # (p, qb, oS) awaiting normalization
            prev_acc = None   # previous pair's accumulator + identity
            for qb in range(QB):
                for p in range(2):
                    o_ps = ps.tile([P, 1024], f32, tag="acc", bufs=1,
                                   name="o_ps")
                    for kt in range(KT):
                        here = fills.get((qb, p, kt), [])
                        for fn in here:
                            fn()
                        if prev is not None and kt in (7, 8):
                            rbm(prev[0], prev[1], prev[2], kt - 7)
                        scores_exp(qb, p, kt, pend, o_ps)
                        if kt == 0 and prev_acc is not None:
                            # the previous pair's remaining PV + evacuation is
                            # emitted after this pair's first scores/exp so the
                            # Scalar engine keeps streaming over the boundary
                            pp, pqb, po = prev_acc
                            while pend and pend[0][2] is po:
                                pv(pend)
                            prev = (pp, pqb, evac(po))
                        while len(pend) > SKEW:
                            pv(pend)
                    prev_acc = (p, qb, o_ps)
            # tail: drain the last pair, then the final out-projection as
            # four double-buffered (128-token, full-width) groups.
            while pend:
                pv(pend)
            oS = evac(o_ps, eng=nc.scalar)
            rbm(prev_acc[0], prev_acc[1], oS, 0)
            rbm(prev_acc[0], prev_acc[1], oS, 1)
            for qt in range(4):
                tok = slice((QB - 1) * 512 + qt * 128,
                            (QB - 1) * 512 + (qt + 1) * 128)
                y_ps = ps.tile([P, 1024], f32, tag="s", bufs=2, name="s_ps")
                for n2 in range(2):
                    for ks2 in range(2):
                        nc.tensor.matmul(
                            y_ps[:, n2 * 512:(n2 + 1) * 512],
                            lhsT=oT_sb[:, ks2, tok],
                            rhs=wo_sb[:, ks2, n2 * 512:(n2 + 1) * 512],
                            start=(ks2 == 0), stop=(ks2 == 1),
                        )
                y_sb = sb.tile([P, 1024], bf16, tag="ysb2", bufs=2, name="y_sb2")
                with nc.allow_low_precision(reason="partial y summed on host"):
                    nc.vector.tensor_copy(y_sb, y_ps)
                nc.sync.dma_start(out_y[tok, :], y_sb)

    nc.compile()
    return nc


def _get_nc():
    if "nc" not in _cache:
        _cache["nc"] = _build_nc()
    return _cache["nc"]


def kernel(x, W_in, b_in, W_out, b_out):
    import ml_dtypes
    from concourse.bass_utils import run_bass_kernel_spmd

    bf16 = ml_dtypes.bfloat16
    x = np.asarray(x, dtype=np.float32)
    W_in = np.asarray(W_in, dtype=np.float32)
    b_in = np.asarray(b_in, dtype=np.float32)
    W_out = np.asarray(W_out, dtype=np.float32)
    b_out = np.asarray(b_out, dtype=np.float32)

    in_maps = []
    for c in range(8):
        b, g = c // 4, c % 4
        rs = slice(256 * g, 256 * g + 256)

        xTc = np.ascontiguousarray(
            x[b].T.reshape(8, 128, N).transpose(1, 0, 2)).astype(bf16)
        Wqk = np.concatenate([W_in[0:C][rs], W_in[C:2 * C][rs]])   # [512,1024]
        wqkc = np.ascontiguousarray(
            Wqk.T.reshape(8, 128, 512).transpose(1, 0, 2)).astype(bf16)
        Wv = W_in[2 * C:3 * C][rs]                                 # [256,1024]
        wvc = np.ascontiguousarray(
            Wv.T.reshape(8, 128, 256).transpose(1, 0, 2)).astype(bf16)
        WoT = np.ascontiguousarray(W_out[:, rs].T)                 # [256,1024]
        woc = np.ascontiguousarray(
            WoT.reshape(2, 128, 1024).transpose(1, 0, 2)).astype(bf16)
        bqkc = np.ascontiguousarray(
            np.concatenate([b_in[0:C][rs], b_in[C:2 * C][rs]]).reshape(4, 128).T)

        in_maps.append({"xT": xTc, "wqk": wqkc, "wv": wvc, "wo": woc, "bqk": bqkc})

    nc = _get_nc()
    trace = os.environ.get("KERNEL_TRACE", "0") == "1"
    bkr = run_bass_kernel_spmd(nc, in_maps, core_ids=list(range(8)), trace=trace)
    _cache["last_bkr"] = bkr
    res = bkr.results

    y = np.zeros((B, N, C), dtype=np.float32)
    for c in range(8):
        y[c // 4] += res[c]["out_y"]
    # v-bias folds through softmax (rows sum to 1) and out-proj exactly
    y += (b_in[2 * C:3 * C] @ W_out.T + b_out)[None, None, :]
    return y
